# revision 23
# baseline (speedup 1.0000x reference)
"""GNN message passing (segment_sum of gathered node features) on 8 TRN2 cores.

Strategy (node-sharded CSR gather, fp16 data path):
  h[r] = sum_{e: row_e == r} x[col_e]
- Host: compute in-degree of every node, sort nodes by degree (desc), tile
  into 80 groups of 128 nodes, assign 10 tiles to each of the 8 cores so that
  same-slot tiles across cores have matching max degree (adjacent in sorted
  order). Each node's neighbor (col) list is padded to the slot max degree
  K_s with index N (a zero row of the padded x).
- Device (per core, SPMD): for each of its 10 slots: chunked dma_gather
  ucode ops pull K_s*128 rows of 256B (fp16) each from DRAM into SBUF
  laid out [128 nodes x K_s*128 feats]; DVE tree-halving adds (2x perf
  mode on fp16) reduce to [128,128]; ACT casts to fp32; HWDGE DMA result
  out. No collectives; host concatenates + inverse-permutes the 8 shards.
- fp16 halves gather bytes and doubles DVE add throughput; max rel err vs
  the fp32 reference is ~1.1e-3 (gate is 2e-2). Chunk size 12 blocks keeps
  each gather op's descriptor burst (1536 descs) exactly inside the 96 KiB
  SWDGE scratch ring; larger chunks measured slower (ring overflow stalls).
- A `reps` input drives a dynamic For_i hardware loop around the whole
  body so test harnesses can measure per-iteration HW time by differencing
  wall time at reps=1 vs reps=R (tunnel/upload overhead cancels). The
  graded path runs reps=1.
"""

import os
import numpy as np

N = 10000
F = 128
E = 640000
NCORES = 8
P = 128
NT = 80          # node tiles of 128 (10240 padded nodes)
S = NT // NCORES  # tile slots per core

_PROG_CACHE = {}


def _dtype_np():
    return np.float32 if os.environ.get("KDTYPE", "f16") == "f32" else np.float16


def _prep(x, edge_index):
    dt = _dtype_np()
    x = np.ascontiguousarray(np.asarray(x, dtype=np.float32))
    ei = np.asarray(edge_index)
    row = ei[0].astype(np.int64)
    col = ei[1].astype(np.int64)

    deg = np.bincount(row, minlength=N)

    # nodes ordered by degree desc; stable for reproducibility
    order = np.argsort(-deg, kind="stable")
    deg_sorted = deg[order]

    # CSR of incoming neighbors, grouped by destination row
    eorder = np.argsort(row, kind="stable")
    scol = col[eorder].astype(np.int16)
    indptr = np.zeros(N + 1, dtype=np.int64)
    indptr[1:] = np.cumsum(deg)

    # slot max degrees: tiles 8s..8s+7 in slot s; degrees are non-increasing
    # along the sorted order so the first node of tile 8s has the slot max.
    Ks = []
    for s in range(S):
        q = s * NCORES * P
        Ks.append(max(int(deg_sorted[q]) if q < N else 0, 1))
    Kmax = max(Ks)

    # padded neighbor table [NT*P, Kmax]; pad index N -> appended zero row
    karange = np.arange(Kmax)
    startp = indptr[order]
    mask = karange[None, :] < deg_sorted[:, None]
    src = np.minimum(startp[:, None] + karange[None, :], E - 1)
    tbl_full = np.full((NT * P, Kmax), N, dtype=np.int16)
    tbl_full[:N] = np.where(mask, scol[src], np.int16(N))

    # per-core wrapped index tensors (k-major within each slot tile)
    idx_cores = []
    for c in range(NCORES):
        blocks = []
        for s in range(S):
            t = s * NCORES + c
            blk = tbl_full[t * P : (t + 1) * P, : Ks[s]].T  # [K, P]
            idx_lin = np.ascontiguousarray(blk).reshape(-1)  # i = k*128 + p
            w = idx_lin.reshape(-1, 16)                      # [n/16, 16]
            sb = np.tile(w.T, (8, 1))                        # [128, n/16] replicated x8
            blocks.append(sb)
        idx_cores.append(np.ascontiguousarray(np.concatenate(blocks, axis=1)))

    # x padded to NT*P rows; row N (zeros) is the target of padding indices
    xpad = np.zeros((NT * P, F), dtype=dt)
    xpad[:N] = x.astype(dt)

    return xpad, idx_cores, Ks, order


def _build_program(Ks):
    import concourse.bass as bass
    import concourse.tile as tile
    from concourse import bacc, mybir

    dt = mybir.dt.float32 if os.environ.get("KDTYPE", "f16") == "f32" else mybir.dt.float16

    s_total = sum(K * P // 16 for K in Ks)

    nc = bacc.Bacc(
        "TRN2",
        target_bir_lowering=False,
        debug=False,
        num_devices=NCORES,
        dynamic_dma_scratch_size=int(os.environ.get("KSCRATCH", "98304")),
        num_swdge_queues=int(os.environ.get("KNQ", "4")),
    )
    _SRC_SBUF = os.environ.get("KSRC", "hbm") == "sbuf"
    NXROWS = NT * P  # x padded to 10240 rows; row N (=10000) is the zero row
    xp = nc.declare_dram_parameter("xp", [NXROWS, F], dt, isOutput=False)
    idx = nc.declare_dram_parameter("idx", [128, s_total], mybir.dt.int16, isOutput=False)
    reps = nc.declare_dram_parameter("reps", [1, 1], mybir.dt.int32, isOutput=False)
    out = nc.declare_dram_parameter("out", [S * P, F], mybir.dt.float32, isOutput=True)

    from contextlib import ExitStack

    CHUNK = int(os.environ.get("KCHUNK", "12"))  # blocks of 128 idx per dma_gather

    with tile.TileContext(nc) as tc:
        with ExitStack() as ctx:
            _BUFS = int(os.environ.get("KBUFS", "10"))
            ipool = ctx.enter_context(tc.tile_pool(name="idx", bufs=1))
            rpool = ctx.enter_context(tc.tile_pool(name="reps", bufs=1))
            gpool = ctx.enter_context(tc.tile_pool(name="gath", bufs=_BUFS))
            apool = ctx.enter_context(tc.tile_pool(name="acc", bufs=min(_BUFS, 2)))
            opool = ctx.enter_context(tc.tile_pool(name="outc", bufs=2))

            # load the whole wrapped index array into SBUF once
            it_all = ipool.tile([128, s_total], mybir.dt.int16)
            nc.sync.dma_start(it_all[:], idx[:, :])

            xsb = None
            if _SRC_SBUF:
                # resident copy of x: node s*128+p -> partition p, stripe s
                xsb = ipool.tile([128, (NXROWS // P) * F], dt)
                nc.sync.dma_start(
                    xsb[:].rearrange("p (s f) -> p s f", f=F),
                    xp[:, :].rearrange("(s p) f -> p s f", p=P),
                )

            _STATIC = os.environ.get("KSTATIC", "0") == "1"
            if not _STATIC:
                rtile = rpool.tile([1, 1], mybir.dt.int32)
                nc.sync.dma_start(rtile[:], reps[:, :])
                reps_val = nc.values_load(
                    rtile[0:1, 0:1], min_val=1, max_val=1 << 20,
                    skip_runtime_bounds_check=True,
                )

            _NQ = int(os.environ.get("KNQ", "4"))
            _gq = [0]
            _NSLOTS = int(os.environ.get("KSLOTS", str(S)))
            _NOADD = os.environ.get("KNOADD", "0") == "1"
            _NOGATHER = os.environ.get("KNOGATHER", "0") == "1"

            from contextlib import nullcontext
            with (nullcontext() if _STATIC else tc.For_i(0, reps_val)):
                off = 0  # col offset into idx (wrapped: block j -> cols j*8..j*8+7)
                for s in range(_NSLOTS):
                    K = Ks[s]
                    nchunks = (K + CHUNK - 1) // CHUNK
                    if nchunks > 1:
                        acc = apool.tile([128, F], dt, tag="acc")
                    else:
                        acc = None
                    # equalize chunk sizes (avoid tiny tail gathers)
                    base, extra = divmod(K, nchunks)
                    bounds = [0]
                    for c in range(nchunks):
                        bounds.append(bounds[-1] + base + (1 if c < extra else 0))
                    for c in range(nchunks):
                        j0, j1 = bounds[c], bounds[c + 1]
                        W = j1 - j0
                        n = W * P
                        g = gpool.tile([128, n], dt, tag="g")
                        if not _NOGATHER:
                            if _SRC_SBUF:
                                # transpose-mode gather from resident x:
                                # g[f, i] = x[idx_i, f] (feature-major)
                                nc.gpsimd.dma_gather(
                                    g[:].rearrange("p (one n) -> p one n", one=1),
                                    xsb[:],
                                    it_all[:, off + j0 * 8 : off + j1 * 8],
                                    num_idxs=n,
                                    num_idxs_reg=n,
                                    elem_size=F,
                                    transpose=True,
                                    single_packet=False,
                                    queue_num=_gq[0],
                                    sbuf_tokens_per_rank=P,
                                    sbuf_free_dim_per_rank=F * mybir.dt.size(dt),
                                )
                            else:
                                nc.gpsimd.dma_gather(
                                    g[:].rearrange("p (k f) -> p k f", f=F),
                                    xp[:, :],
                                    it_all[:, off + j0 * 8 : off + j1 * 8],
                                    num_idxs=n,
                                    num_idxs_reg=n,
                                    elem_size=F,
                                    single_packet=os.environ.get("KSP", "0") == "1",
                                    queue_num=_gq[0],
                                )
                            _gq[0] = (_gq[0] + 1) % _NQ

                        if _NOADD:
                            W = 1
                        # in-place tree reduction over the W feature blocks
                        while W > 1:
                            half = W // 2
                            nc.vector.tensor_add(
                                out=g[:, 0 : half * F],
                                in0=g[:, 0 : half * F],
                                in1=g[:, (W - half) * F : W * F],
                            )
                            W -= half
                        red = g
                        if nchunks == 1:
                            acc_ap = red[:, 0:F]  # single chunk: reduce IS the slot sum
                        elif c == 0:
                            nc.vector.tensor_copy(acc[:], red[:, 0:F])
                            acc_ap = acc[:]
                        else:
                            nc.vector.tensor_add(out=acc[:], in0=acc[:], in1=red[:, 0:F])
                            acc_ap = acc[:]

                    if dt == mybir.dt.float32:
                        nc.sync.dma_start(out[s * P : (s + 1) * P, :], acc_ap)
                    else:
                        accf = opool.tile([128, F], mybir.dt.float32, tag="accf")
                        nc.scalar.copy(accf[:], acc_ap)
                        nc.sync.dma_start(out[s * P : (s + 1) * P, :], accf[:])
                    off += K * 8

    nc.finalize()
    return nc


def _get_program(Ks):
    key = (
        tuple(Ks),
        tuple(os.environ.get(k, "") for k in (
            "KCHUNK", "KBUFS", "KSCRATCH", "KNQ", "KDTYPE",
            "KSLOTS", "KNOADD", "KNOGATHER", "KSTATIC", "KSRC", "KSP")),
    )
    if key not in _PROG_CACHE:
        _PROG_CACHE[key] = _build_program(Ks)
    return _PROG_CACHE[key]


def _unshard(res, order):
    transposed = os.environ.get("KSRC", "hbm") == "sbuf"
    h = np.zeros((N, F), dtype=np.float32)
    for c in range(NCORES):
        oc = np.asarray(res[c]["out"])
        if transposed:
            # device wrote [f, node] blocks per slot
            oc = np.ascontiguousarray(
                oc.reshape(S, P, F).transpose(0, 2, 1)
            ).reshape(S * P, F)
        for s in range(S):
            t = s * NCORES + c
            lo, hi = t * P, (t + 1) * P
            if lo >= N:
                continue
            vs = order[lo:min(hi, N)]
            h[vs] = oc[s * P : s * P + len(vs)]
    return h


def kernel(x, edge_index):
    from concourse.bass_utils import run_bass_kernel_spmd

    xpad, idx_cores, Ks, order = _prep(x, edge_index)
    nc = _get_program(Ks)

    reps1 = np.ones((1, 1), dtype=np.int32)
    in_maps = [
        {"xp": xpad, "idx": idx_cores[c], "reps": reps1} for c in range(NCORES)
    ]
    try:
        res = run_bass_kernel_spmd(nc, in_maps, list(range(NCORES)))
    except Exception:
        # fall back to a conservative configuration (smaller gathers,
        # shallow pipelining) in case the tuned one trips the SWDGE ring
        os.environ["KCHUNK"] = "8"
        os.environ["KBUFS"] = "2"
        os.environ["KNQ"] = "1"
        os.environ["KSCRATCH"] = "16384"
        nc = _build_program(Ks)
        res = run_bass_kernel_spmd(nc, in_maps, list(range(NCORES)))
    global LAST_RESULT
    LAST_RESULT = res

    return _unshard(res.results, order)
